# revision 1
# baseline (speedup 1.0000x reference)
"""DiT dual-stream attention (B=4, S=2048, D=1024, H=16, DK=DV=64) on 8 TRN2 cores.

Sharding: core i handles batch b = i//2 and head-group g = i%2 (8 heads each).
v2: phases 2+3 merged with qp-outer loop; projection and the pairwise
AllReduce run per 512-token slab, overlapped with the next slab's attention.
All matmuls in float32r (full PE rate, ~1.5e-4 rel err measured on HW).
"""

import os
import sys

for _p in ("/opt/trn_rl_repo", "/root/.axon_site/_ro/trn_rl_repo"):
    if os.path.isdir(_p) and _p not in sys.path:
        sys.path.insert(0, _p)

import numpy as np

import concourse.bass as bass
import concourse.tile as tile
from concourse import bacc, mybir


F32 = mybir.dt.float32
F32R = mybir.dt.float32r

N_CORES = 8
B, S, D = 4, 2048, 1024
H, DK, DV = 16, 64, 64
HL = 8          # local heads per core
FQK = HL * DK   # 512: local q/k width per stream (x or c)
NJ = D // 128   # 8 contraction d-tiles
NP = 4          # token panels of 512
PAN = S // NP   # 512
NT = S // 128   # 16 token tiles
SCALE = 1.0 / np.sqrt(np.float32(DK))


def _build_nc(reps=1):
    nc = bacc.Bacc("TRN2", num_devices=N_CORES)

    xt_in = nc.dram_tensor("xt", [D, S], F32R, kind="ExternalInput")
    ct_in = nc.dram_tensor("ct", [D, S], F32R, kind="ExternalInput")
    w_names = ["wqx", "wqc", "wkx", "wkc", "wvx", "wvc"]
    w_in = {n: nc.dram_tensor(n, [D, FQK], F32R, kind="ExternalInput") for n in w_names}
    wp_in = nc.dram_tensor("wp", [HL * 2 * DV, D], F32R, kind="ExternalInput")
    bias_in = nc.dram_tensor("bias", [D], F32, kind="ExternalInput")
    y_out = nc.dram_tensor("y", [S, D], F32, kind="ExternalOutput")

    # DRAM scratch
    qT_d = nc.dram_tensor("qT_d", [HL, 128, S], F32R)
    kT_d = nc.dram_tensor("kT_d", [HL, 128, S], F32R)
    v_d = nc.dram_tensor("v_d", [NT, 128, HL, 128], F32R)  # [ktile, tok, h, vd]
    # per-slab partial/reduced tensors (separate tensors -> no false deps)
    y_part = [nc.dram_tensor(f"y_part{i}", [PAN, D], F32) for i in range(NP)]
    y_red = [nc.dram_tensor(f"y_red{i}", [PAN, D], F32) for i in range(NP)]

    import contextlib

    with tile.TileContext(nc) as tc:
        for _rep in range(reps):
            with tc.tile_pool(name="consts", bufs=1) as consts:
                ones_f = consts.tile([128, 128], F32)
                nc.vector.memset(ones_f, 1.0)
                ones = consts.tile([128, 128], F32R)
                nc.vector.tensor_copy(ones, ones_f)

                # ---------------- phase 1: QKV projections ----------------
                with (
                    tc.tile_pool(name="wpool", bufs=1) as wpool,
                    tc.tile_pool(name="xtp", bufs=2) as xtp,
                    tc.tile_pool(name="stage1", bufs=4) as stage1,
                    tc.tile_pool(name="ps_mm", bufs=4, space="PSUM") as ps_mm,
                ):
                    w_sb = {}
                    # first q/k weights + panel 0 activations lead the DMA queue
                    for n in ("wqx", "wqc"):
                        w_sb[n] = wpool.tile(
                            [128, NJ, FQK], F32R, tag=f"w_{n}", name=f"w_{n}"
                        )
                        nc.sync.dma_start(
                            out=w_sb[n],
                            in_=w_in[n].rearrange("(j p) f -> p j f", p=128),
                        )
                    panels = {}
                    for p in range(NP):
                        xT = xtp.tile([128, NJ, PAN], F32R, tag="xT", name=f"xT{p}")
                        cT = xtp.tile([128, NJ, PAN], F32R, tag="cT", name=f"cT{p}")
                        panels[p] = (xT, cT)
                        if p == 0:
                            for src, dst in ((xt_in, xT), (ct_in, cT)):
                                nc.sync.dma_start(
                                    out=dst,
                                    in_=src.rearrange("(j p) t -> p j t", p=128)[
                                        :, :, 0:PAN
                                    ],
                                )
                    for n in ("wkx", "wkc", "wvx", "wvc"):
                        w_sb[n] = wpool.tile(
                            [128, NJ, FQK], F32R, tag=f"w_{n}", name=f"w_{n}"
                        )
                        nc.sync.dma_start(
                            out=w_sb[n],
                            in_=w_in[n].rearrange("(j p) f -> p j f", p=128),
                        )

                    for p in range(NP):
                        xT, cT = panels[p]
                        if p > 0:
                            for src, dst in ((xt_in, xT), (ct_in, cT)):
                                nc.sync.dma_start(
                                    out=dst,
                                    in_=src.rearrange("(j p) t -> p j t", p=128)[
                                        :, :, p * PAN : (p + 1) * PAN
                                    ],
                                )

                        # q/k -> [f, tok] layout, scattered into qT_d/kT_d
                        for wn, src, dst_d, poff in (
                            ("wqx", xT, qT_d, 0),
                            ("wqc", cT, qT_d, 64),
                            ("wkx", xT, kT_d, 0),
                            ("wkc", cT, kT_d, 64),
                        ):
                            for fi in range(4):
                                ps = ps_mm.tile([128, PAN], F32, tag="ps1", name="ps")
                                for j in range(NJ):
                                    nc.tensor.matmul(
                                        ps,
                                        w_sb[wn][:, j, fi * 128 : (fi + 1) * 128],
                                        src[:, j, :],
                                        start=(j == 0),
                                        stop=(j == NJ - 1),
                                    )
                                st = stage1.tile([128, PAN], F32R, tag="qkstage")
                                nc.vector.tensor_copy(st, ps)
                                nc.sync.dma_start(
                                    out=dst_d[
                                        2 * fi, poff : poff + 64, p * PAN : (p + 1) * PAN
                                    ],
                                    in_=st[0:64, :],
                                )
                                nc.sync.dma_start(
                                    out=dst_d[
                                        2 * fi + 1,
                                        poff : poff + 64,
                                        p * PAN : (p + 1) * PAN,
                                    ],
                                    in_=st[64:128, :],
                                )

                        # v -> [tok, f] layout
                        for wn, src, voff in (("wvx", xT, 0), ("wvc", cT, 64)):
                            for tt in range(4):
                                ps = ps_mm.tile([128, FQK], F32, tag="ps1", name="ps")
                                for j in range(NJ):
                                    nc.tensor.matmul(
                                        ps,
                                        src[:, j, tt * 128 : (tt + 1) * 128],
                                        w_sb[wn][:, j, :],
                                        start=(j == 0),
                                        stop=(j == NJ - 1),
                                    )
                                st = stage1.tile([128, FQK], F32R, tag="vstage")
                                nc.vector.tensor_copy(st, ps)
                                nc.sync.dma_start(
                                    out=v_d[p * 4 + tt, :, :, voff : voff + 64],
                                    in_=st.rearrange("p (h d) -> p h d", h=HL),
                                )

                # -------- phases 2+3 merged: attention + proj, slab-pipelined --------
                with (
                    tc.tile_pool(name="kres", bufs=1) as kres,
                    tc.tile_pool(name="wp3", bufs=1) as wp3,
                    tc.tile_pool(name="vstr", bufs=2) as vstr,
                    tc.tile_pool(name="qstr", bufs=3) as qstr,
                    tc.tile_pool(name="aores", bufs=1) as aores,
                    tc.tile_pool(name="expp", bufs=6) as expp,
                    tc.tile_pool(name="small2", bufs=3) as small2,
                    tc.tile_pool(name="accp", bufs=2) as accp,
                    tc.tile_pool(name="y3", bufs=4) as y3,
                    tc.tile_pool(name="ps_s", bufs=4, space="PSUM") as ps_s,
                    tc.tile_pool(name="ps_sum", bufs=1, space="PSUM") as ps_sum,
                    tc.tile_pool(name="ps_out", bufs=2, space="PSUM") as ps_out,
                    tc.tile_pool(name="ps_y", bufs=1, space="PSUM") as ps_y,
                ):
                    # resident: all-head K (64KB/part), W_proj (32KB/part), bias
                    k_sb = kres.tile([128, HL, S], F32R, tag="k_sb")
                    for h in range(HL):
                        nc.sync.dma_start(out=k_sb[:, h, :], in_=kT_d[h])

                    wp_sb = wp3.tile([128, HL, D], F32R, tag="wp_sb")
                    nc.sync.dma_start(
                        out=wp_sb, in_=wp_in.rearrange("(j p) f -> p j f", p=128)
                    )

                    bias_b = wp3.tile([128, D], F32, tag="bias_b")
                    b_ap = bias_in[:]
                    nc.sync.dma_start(
                        out=bias_b,
                        in_=bass.AP(
                            tensor=b_ap.tensor,
                            offset=b_ap.offset,
                            ap=[[0, 128]] + [list(p) for p in b_ap.ap],
                        ),
                    )

                    for qp in range(NP):
                        ao_sb = aores.tile([128, HL, PAN], F32R, tag="ao_sb")
                        for h in range(HL):
                            qh = qstr.tile([128, PAN], F32R, tag="qh")
                            nc.sync.dma_start(
                                out=qh, in_=qT_d[h, :, qp * PAN : (qp + 1) * PAN]
                            )
                            vh = vstr.tile([128, NT, 128], F32R, tag="vh")
                            nc.sync.dma_start(
                                out=vh, in_=v_d[:, :, h, :].rearrange("k p d -> p k d")
                            )
                            p_sum = ps_sum.tile([128, PAN], F32, tag="p_sum")
                            p_out = ps_out.tile([128, PAN], F32, tag="p_out")
                            # exp tiles are summed over k-tiles on DVE (f32);
                            # one ones-matmul on the accumulated tile then
                            # partition-sums the final [128, PAN] (saves 15/16
                            # of the denominator matmul work on PE)
                            acc = accp.tile([128, PAN], F32, tag="acc")
                            acc_r = accp.tile([128, PAN], F32R, tag="acc_r")
                            ex0 = None
                            for kt in range(NT):
                                p_s = ps_s.tile([128, PAN], F32, tag="p_s")
                                nc.tensor.matmul(
                                    p_s,
                                    k_sb[:, h, kt * 128 : (kt + 1) * 128],
                                    qh,
                                    start=True,
                                    stop=True,
                                )
                                ex = expp.tile([128, PAN], F32R, tag="ex")
                                nc.scalar.activation(
                                    out=ex,
                                    in_=p_s,
                                    func=mybir.ActivationFunctionType.Exp,
                                    scale=float(SCALE),
                                )
                                if kt == 0:
                                    ex0 = ex
                                elif kt == 1:
                                    nc.vector.tensor_add(
                                        acc, ex0.bitcast(F32), ex.bitcast(F32)
                                    )
                                elif kt < NT - 1:
                                    nc.vector.tensor_add(acc, acc, ex.bitcast(F32))
                                else:
                                    nc.vector.tensor_add(acc_r, acc, ex.bitcast(F32))
                                nc.tensor.matmul(
                                    p_out,
                                    vh[:, kt, :],
                                    ex,
                                    start=(kt == 0),
                                    stop=(kt == NT - 1),
                                )
                            nc.tensor.matmul(p_sum, ones, acc_r, start=True, stop=True)
                            inv = small2.tile([128, PAN], F32, tag="inv")
                            nc.vector.reciprocal(inv, p_sum)
                            nc.vector.tensor_mul(ao_sb[:, h, :], p_out, inv)

                        # projection for this 512-token slab
                        for tt4 in range(4):
                            for do in range(2):
                                ps = ps_y.tile([128, 512], F32, tag="ps_y")
                                for fi in range(HL):
                                    nc.tensor.matmul(
                                        ps,
                                        ao_sb[:, fi, tt4 * 128 : (tt4 + 1) * 128],
                                        wp_sb[:, fi, do * 512 : (do + 1) * 512],
                                        start=(fi == 0),
                                        stop=(fi == HL - 1),
                                    )
                                yt = y3.tile([128, 512], F32, tag="yt")
                                nc.vector.tensor_add(
                                    yt, ps, bias_b[:, do * 512 : (do + 1) * 512]
                                )
                                nc.sync.dma_start(
                                    out=y_part[qp][
                                        tt4 * 128 : (tt4 + 1) * 128,
                                        do * 512 : (do + 1) * 512,
                                    ],
                                    in_=yt,
                                )

                        # pairwise all-reduce of this slab, overlapped with next slab
                        nc.gpsimd.collective_compute(
                            "AllReduce",
                            mybir.AluOpType.add,
                            replica_groups=[[0, 1], [2, 3], [4, 5], [6, 7]],
                            ins=[y_part[qp][:]],
                            outs=[y_red[qp][:]],
                        )
                        nc.gpsimd.dma_start(
                            out=y_out[qp * PAN : (qp + 1) * PAN, :], in_=y_red[qp][:]
                        )

    nc.finalize()
    return nc


_NC = {}


def _get_nc(reps=1):
    global _NC
    if _NC is None:
        _NC = {}
    if reps not in _NC:
        _NC[reps] = _build_nc(reps)
    return _NC[reps]


def _shard_inputs(inputs):
    x = np.ascontiguousarray(inputs["x"], dtype=np.float32)
    c = np.ascontiguousarray(inputs["c"], dtype=np.float32)
    wq_x, wk_x, wv_x = inputs["Wq_x"], inputs["Wk_x"], inputs["Wv_x"]
    wq_c, wk_c, wv_c = inputs["Wq_c"], inputs["Wk_c"], inputs["Wv_c"]
    w_proj, b_proj = inputs["W_proj"], inputs["b_proj"]

    in_maps = []
    for core in range(N_CORES):
        b, g = core // 2, core % 2
        fs = slice(g * FQK, (g + 1) * FQK)
        m = {
            "xt": np.ascontiguousarray(x[b].T),
            "ct": np.ascontiguousarray(c[b].T),
            "wqx": np.ascontiguousarray(wq_x[:, fs], dtype=np.float32),
            "wqc": np.ascontiguousarray(wq_c[:, fs], dtype=np.float32),
            "wkx": np.ascontiguousarray(wk_x[:, fs], dtype=np.float32),
            "wkc": np.ascontiguousarray(wk_c[:, fs], dtype=np.float32),
            "wvx": np.ascontiguousarray(wv_x[:, fs], dtype=np.float32),
            "wvc": np.ascontiguousarray(wv_c[:, fs], dtype=np.float32),
            "wp": np.ascontiguousarray(
                w_proj[g * HL * 2 * DV : (g + 1) * HL * 2 * DV, :], dtype=np.float32
            ),
            "bias": (
                np.ascontiguousarray(b_proj, dtype=np.float32)
                if g == 0
                else np.zeros((D,), np.float32)
            ),
        }
        in_maps.append(m)
    return in_maps


def kernel(**inputs) -> np.ndarray:
    from concourse.bass_utils import run_bass_kernel_spmd

    nc = _get_nc()
    in_maps = _shard_inputs(inputs)
    res = run_bass_kernel_spmd(nc, in_maps, list(range(N_CORES)))
    y = np.stack([res.results[2 * b]["y"] for b in range(B)], axis=0)
    return y.astype(np.float32)



# revision 2
# speedup vs baseline: 1.0298x; 1.0298x over previous
"""DiT dual-stream attention (B=4, S=2048, D=1024, H=16, DK=DV=64) on 8 TRN2 cores.

Sharding: core i handles batch b = i//2 and head-group g = i%2 (8 heads each).
v6 (final): single-pass bf16 phase 1 with QKV written straight into SBUF residents.
DMA trigger count minimized (the Sync engine issues triggers serially at
~0.8us each): whole-tensor panel loads, 4-head-batched K/Q SBUF moves into a
[128, 2, 4, S] resident layout, y traffic on the GpSimd queue. Phase 2: exp
over [128,1024] spanning two PSUM banks, bf16 pair-sum tree + 4 ones-matmuls
for the softmax denominator, reciprocal_approx_fast, projection of slab i
interleaved into slab i+1, pairwise ReduceScatter per 256-token chunk, and
uneven slabs (512,512,512,256,256) to shrink the un-overlapped tail.
"""

import os
import sys

for _p in ("/opt/trn_rl_repo", "/root/.axon_site/_ro/trn_rl_repo"):
    if os.path.isdir(_p) and _p not in sys.path:
        sys.path.insert(0, _p)

import numpy as np

import concourse.bass as bass
import concourse.tile as tile
from concourse import bacc, mybir


F32 = mybir.dt.float32
F32R = mybir.dt.float32r
BF16 = mybir.dt.bfloat16

N_CORES = 8
B, S, D = 4, 2048, 1024
H, DK, DV = 16, 64, 64
HL = 8          # local heads per core
FQK = HL * DK   # 512: local q/k width per stream (x or c)
NJ = D // 128   # 8 contraction d-tiles
P1N = 8         # phase-1 token panels
P1PAN = S // P1N  # 512
NT = S // 128   # 16 kv token tiles
PAN = 512       # max slab width (tile allocation size)
# phase-2 slabs: (token base, width)
SLABS = [(0, 512), (512, 512), (1024, 512), (1536, 256), (1792, 128), (1920, 128)]
SCALE = 1.0 / np.sqrt(np.float32(DK))


def _n_groups(w):
    return (w // 128) * 2


def _build_nc(reps=1):
    nc = bacc.Bacc("TRN2", num_devices=N_CORES)

    xt_in = nc.dram_tensor("xt", [D, S], F32R, kind="ExternalInput")
    ct_in = nc.dram_tensor("ct", [D, S], F32R, kind="ExternalInput")
    w_names = ["wqx", "wqc", "wkx", "wkc", "wvx", "wvc"]
    w_in = {n: nc.dram_tensor(n, [D, FQK], F32R, kind="ExternalInput") for n in w_names}
    wp_in = nc.dram_tensor("wp", [HL * 2 * DV, D], F32R, kind="ExternalInput")
    bias_in = nc.dram_tensor("bias", [D], F32, kind="ExternalInput")
    y_out = nc.dram_tensor("y", [S, D], F32, kind="ExternalOutput")

    # per-slab partial tensors + per-256-chunk reduced tensors
    y_part = [nc.dram_tensor(f"y_part{i}", [PAN, D], F32) for i in range(len(SLABS))]
    y_red = [
        [
            nc.dram_tensor(f"y_red{i}h{j}", [min(w, 256) // 2, D], F32)
            for j in range(max(1, w // 256))
        ]
        for i, (_, w) in enumerate(SLABS)
    ]

    with tile.TileContext(nc) as tc:
        for _rep in range(reps):
            with tc.tile_pool(name="resident", bufs=1) as res:
                # bf16 resident K/Q: [2DK=128, half, fi, S]; head h = 2*fi + half
                # x-stream in partitions 0:64, c-stream in 64:128
                k_one = res.tile([128, 2, 4, S], BF16, tag="k_one")
                q_one = res.tile([128, 2, 4, S], BF16, tag="q_one")
                wp_sb = res.tile([128, HL, D], BF16, tag="wp_sb")

                def k_h(h):
                    return k_one[:, h % 2, h // 2, :]

                def q_h(h):
                    return q_one[:, h % 2, h // 2, :]

                with tc.tile_pool(name="vres", bufs=1) as vres:
                    # [tok-in-tile, kt, head, vd]; x-stream vd 0:64, c 64:128
                    v_sb = vres.tile([128, NT, HL, 128], BF16, tag="v_sb")

                    # ---------------- phase 1: QKV single pass (bf16) ----------
                    with (
                        tc.tile_pool(name="wpool", bufs=1) as wpool,
                        tc.tile_pool(name="wstage", bufs=4) as wstage,
                        tc.tile_pool(name="xtp", bufs=2) as xtp,
                        tc.tile_pool(name="st4p", bufs=2) as st4p,
                        tc.tile_pool(name="ps_qk", bufs=3, space="PSUM") as ps_qk,
                        tc.tile_pool(name="ps_v", bufs=2, space="PSUM") as ps_v,
                    ):
                        w_bf = {}
                        for n in ("wkx", "wkc", "wqx", "wqc", "wvx", "wvc"):
                            w_bf[n] = wpool.tile(
                                [128, NJ, FQK], BF16, tag=f"w_{n}", name=f"w_{n}"
                            )
                        panels = {}
                        for p in range(P1N):
                            xT = xtp.tile([128, NJ, P1PAN], BF16, tag="xT", name=f"xT{p}")
                            cT = xtp.tile([128, NJ, P1PAN], BF16, tag="cT", name=f"cT{p}")
                            panels[p] = (xT, cT)

                        def load_weight(n):
                            # per-j chunks keep the f32 staging footprint small
                            for j in range(NJ):
                                wst = wstage.tile([128, FQK], F32R, tag="wst", name="wst")
                                nc.sync.dma_start(
                                    out=wst,
                                    in_=w_in[n].rearrange("(j p) f -> p j f", p=128)[
                                        :, j, :
                                    ],
                                )
                                nc.vector.tensor_copy(w_bf[n][:, j, :], wst)

                        pstage_pool = [None]

                        def load_panel(p, pstage):
                            xT, cT = panels[p]
                            for src, dst in ((xt_in, xT), (ct_in, cT)):
                                pst = pstage.tile(
                                    [128, NJ, P1PAN], F32R, tag="pst", name="pst"
                                )
                                nc.sync.dma_start(
                                    out=pst,
                                    in_=src.rearrange("(j p) t -> p j t", p=128)[
                                        :, :, p * P1PAN : (p + 1) * P1PAN
                                    ],
                                )
                                nc.vector.tensor_copy(dst, pst)

                        with tc.tile_pool(name="pstage", bufs=2) as pstage:
                            load_weight("wkx")
                            load_panel(0, pstage)
                            for n in ("wkc", "wqx", "wqc", "wvx", "wvc"):
                                load_weight(n)

                            for p in range(P1N):
                                if p + 1 < P1N:
                                    load_panel(p + 1, pstage)
                                xT, cT = panels[p]
                                ts = slice(p * P1PAN, (p + 1) * P1PAN)
                                for wn, src, dst_one, poff in (
                                    ("wkx", xT, k_one, 0),
                                    ("wkc", cT, k_one, 64),
                                    ("wqx", xT, q_one, 0),
                                    ("wqc", cT, q_one, 64),
                                ):
                                    st4 = st4p.tile(
                                        [128, 4, P1PAN], BF16, tag="st4", name="st4"
                                    )
                                    for fi in range(4):
                                        ps = ps_qk.tile(
                                            [128, P1PAN], F32, tag="psA", name="psA"
                                        )
                                        for j in range(NJ):
                                            nc.tensor.matmul(
                                                ps,
                                                w_bf[wn][:, j, fi * 128 : (fi + 1) * 128],
                                                src[:, j, :],
                                                start=(j == 0),
                                                stop=(j == NJ - 1),
                                            )
                                        nc.scalar.activation(
                                            out=st4[:, fi, :],
                                            in_=ps,
                                            func=mybir.ActivationFunctionType.Copy,
                                        )
                                    # heads 2fi (x of st4 rows 0:64) / 2fi+1 (64:128)
                                    nc.sync.dma_start(
                                        out=dst_one[poff : poff + 64, 0, :, ts],
                                        in_=st4[0:64, :, :],
                                    )
                                    nc.sync.dma_start(
                                        out=dst_one[poff : poff + 64, 1, :, ts],
                                        in_=st4[64:128, :, :],
                                    )
                                for wn, src, voff in (("wvx", xT, 0), ("wvc", cT, 64)):
                                    for tt in range(2):
                                        kt = 2 * p + tt
                                        ps = ps_v.tile(
                                            [128, FQK], F32, tag="psB", name="psB"
                                        )
                                        for j in range(NJ):
                                            nc.tensor.matmul(
                                                ps,
                                                src[:, j, tt * 128 : (tt + 1) * 128],
                                                w_bf[wn][:, j, :],
                                                start=(j == 0),
                                                stop=(j == NJ - 1),
                                            )
                                        nc.vector.tensor_copy(
                                            v_sb[:, kt, :, voff : voff + 64],
                                            ps.rearrange("p (h d) -> p h d", h=HL),
                                        )

                    # ---------------- phase 2: attention + proj ----------------
                    with (
                        tc.tile_pool(name="aores", bufs=2) as aores,
                        tc.tile_pool(name="expp", bufs=4) as expp,
                        tc.tile_pool(name="treep", bufs=2) as treep,
                        tc.tile_pool(name="small2", bufs=2) as small2,
                        tc.tile_pool(name="y3", bufs=4) as y3,
                        tc.tile_pool(name="onesp", bufs=1) as onesp,
                        tc.tile_pool(name="wpstage", bufs=2) as wpstage,
                        tc.tile_pool(name="ps_s", bufs=2, space="PSUM") as ps_s,
                        tc.tile_pool(name="ps_sum", bufs=1, space="PSUM") as ps_sum,
                        tc.tile_pool(name="ps_out", bufs=2, space="PSUM") as ps_out,
                        tc.tile_pool(name="ps_y", bufs=1, space="PSUM") as ps_y,
                    ):
                        ones_f = onesp.tile([128, 128], F32, tag="ones_f")
                        ones = onesp.tile([128, 128], BF16, tag="ones")
                        nc.vector.memset(ones_f, 1.0)
                        nc.vector.tensor_copy(ones, ones_f)
                        bias_b = onesp.tile([128, D], F32, tag="bias_b")
                        b_ap = bias_in[:]
                        nc.sync.dma_start(
                            out=bias_b,
                            in_=bass.AP(
                                tensor=b_ap.tensor,
                                offset=b_ap.offset,
                                ap=[[0, 128]] + [list(p) for p in b_ap.ap],
                            ),
                        )
                        for j in range(NJ):
                            wps = wpstage.tile([128, D], F32R, tag="wps", name="wps")
                            nc.sync.dma_start(
                                out=wps,
                                in_=wp_in.rearrange("(j p) f -> p j f", p=128)[:, j, :],
                            )
                            nc.vector.tensor_copy(wp_sb[:, j, :], wps)

                        ao_slabs = [None] * len(SLABS)

                        def emit_proj_group(si, g):
                            # one (tt4, do) output chunk of slab si's projection
                            tt4, do = g // 2, g % 2
                            ao_sb = ao_slabs[si]
                            ps = ps_y.tile([128, 512], F32, tag="ps_y")
                            for fi in range(HL):
                                nc.tensor.matmul(
                                    ps,
                                    ao_sb[:, fi, tt4 * 128 : (tt4 + 1) * 128],
                                    wp_sb[:, fi, do * 512 : (do + 1) * 512],
                                    start=(fi == 0),
                                    stop=(fi == HL - 1),
                                )
                            yt = y3.tile([128, 512], F32, tag="yt")
                            nc.vector.tensor_add(yt, ps, bias_b[:, do * 512 : (do + 1) * 512])
                            nc.sync.dma_start(
                                out=y_part[si][
                                    tt4 * 128 : (tt4 + 1) * 128, do * 512 : (do + 1) * 512
                                ],
                                in_=yt,
                            )

                        def emit_collective(si, ci):
                            # pairwise reduce-scatter of one token chunk
                            base, w = SLABS[si]
                            cs = min(w, 256)
                            off = ci * cs
                            nc.gpsimd.collective_compute(
                                "ReduceScatter",
                                mybir.AluOpType.add,
                                replica_groups=[[0, 1], [2, 3], [4, 5], [6, 7]],
                                ins=[y_part[si][off : off + cs, :]],
                                outs=[y_red[si][ci][:]],
                            )
                            prow = base + off // 2
                            nc.gpsimd.dma_start(
                                out=y_out[prow : prow + cs // 2, :],
                                in_=y_red[si][ci][:],
                            )

                        def _maybe_rs(si, g):
                            cs = min(SLABS[si][1], 256)
                            cg = (cs // 128) * 2  # proj groups per chunk
                            if (g + 1) % cg == 0:
                                emit_collective(si, (g + 1) // cg - 1)

                        def emit_prev_work(si, h):
                            # interleave slab si-1's projection into slab si's heads
                            if si == 0:
                                return
                            ng = _n_groups(SLABS[si - 1][1])
                            if ng == 8:
                                gs = [h]
                            elif ng == 4:
                                gs = [h // 2] if h % 2 == 1 else []
                            else:
                                gs = [h // 4] if h % 4 == 3 else []
                            for g in gs:
                                emit_proj_group(si - 1, g)
                                _maybe_rs(si - 1, g)

                        for si, (base, w) in enumerate(SLABS):
                            ao_sb = aores.tile([128, HL, PAN], BF16, tag="ao_sb")
                            ao_slabs[si] = ao_sb
                            qs = slice(base, base + w)
                            for h in range(HL):
                                p_sum = ps_sum.tile([128, PAN], F32, tag="p_sum")
                                p_out = ps_out.tile([128, PAN], F32, tag="p_out")
                                ex2 = [None] * 8
                                pair = [None] * 4
                                for k2 in range(8):
                                    ps2 = ps_s.tile([128, 2, PAN], F32, tag="p_s")
                                    for i in range(2):
                                        nc.tensor.matmul(
                                            ps2[:, i, 0:w],
                                            k_h(h)[
                                                :, (2 * k2 + i) * 128 : (2 * k2 + i + 1) * 128
                                            ],
                                            q_one[:, h % 2, h // 2, qs],
                                            start=True,
                                            stop=True,
                                        )
                                    ex2[k2] = expp.tile(
                                        [128, 2, PAN], BF16, tag="ex", name="ex"
                                    )
                                    nc.scalar.activation(
                                        out=ex2[k2][:, :, 0:w],
                                        in_=ps2[:, :, 0:w],
                                        func=mybir.ActivationFunctionType.Exp,
                                        scale=float(SCALE),
                                    )
                                    if k2 >= 1:
                                        for i in range(2):
                                            kt = 2 * (k2 - 1) + i
                                            nc.tensor.matmul(
                                                p_out[:, 0:w],
                                                v_sb[:, kt, h, :],
                                                ex2[k2 - 1][:, i, 0:w],
                                                start=(kt == 0),
                                                stop=False,
                                            )
                                    if k2 % 2 == 1:
                                        g = k2 // 2
                                        pair[g] = treep.tile(
                                            [128, 2, PAN], BF16, tag=f"pair{g}",
                                            name=f"pair{g}",
                                        )
                                        nc.vector.tensor_add(
                                            pair[g][:, :, 0:w],
                                            ex2[k2 - 1][:, :, 0:w],
                                            ex2[k2][:, :, 0:w],
                                        )
                                for i in range(2):
                                    kt = 14 + i
                                    nc.tensor.matmul(
                                        p_out[:, 0:w],
                                        v_sb[:, kt, h, :],
                                        ex2[7][:, i, 0:w],
                                        start=False,
                                        stop=(kt == NT - 1),
                                    )
                                nc.vector.tensor_add(
                                    pair[0][:, :, 0:w], pair[0][:, :, 0:w], pair[1][:, :, 0:w]
                                )
                                nc.vector.tensor_add(
                                    pair[2][:, :, 0:w], pair[2][:, :, 0:w], pair[3][:, :, 0:w]
                                )
                                mmi = 0
                                for src_t in (pair[0], pair[2]):
                                    for i in range(2):
                                        nc.tensor.matmul(
                                            p_sum[:, 0:w],
                                            ones,
                                            src_t[:, i, 0:w],
                                            start=(mmi == 0),
                                            stop=(mmi == 3),
                                        )
                                        mmi += 1
                                inv = small2.tile([128, PAN], F32, tag="inv")
                                nc.vector.reciprocal_approx_fast(
                                    inv[:, 0:w], p_sum[:, 0:w]
                                )
                                nc.vector.tensor_mul(
                                    ao_sb[:, h, 0:w], p_out[:, 0:w], inv[:, 0:w]
                                )
                                emit_prev_work(si, h)
                            if si == len(SLABS) - 1:
                                for g in range(_n_groups(w)):
                                    emit_proj_group(si, g)
                                    _maybe_rs(si, g)

    nc.finalize()
    return nc


_NC = {}


def _get_nc(reps=1):
    global _NC
    if _NC is None:
        _NC = {}
    if reps not in _NC:
        _NC[reps] = _build_nc(reps)
    return _NC[reps]


def _shard_inputs(inputs):
    x = np.ascontiguousarray(inputs["x"], dtype=np.float32)
    c = np.ascontiguousarray(inputs["c"], dtype=np.float32)
    wq_x, wk_x, wv_x = inputs["Wq_x"], inputs["Wk_x"], inputs["Wv_x"]
    wq_c, wk_c, wv_c = inputs["Wq_c"], inputs["Wk_c"], inputs["Wv_c"]
    w_proj, b_proj = inputs["W_proj"], inputs["b_proj"]

    in_maps = []
    for core in range(N_CORES):
        b, g = core // 2, core % 2
        fs = slice(g * FQK, (g + 1) * FQK)
        m = {
            "xt": np.ascontiguousarray(x[b].T),
            "ct": np.ascontiguousarray(c[b].T),
            "wqx": np.ascontiguousarray(wq_x[:, fs], dtype=np.float32),
            "wqc": np.ascontiguousarray(wq_c[:, fs], dtype=np.float32),
            "wkx": np.ascontiguousarray(wk_x[:, fs], dtype=np.float32),
            "wkc": np.ascontiguousarray(wk_c[:, fs], dtype=np.float32),
            "wvx": np.ascontiguousarray(wv_x[:, fs], dtype=np.float32),
            "wvc": np.ascontiguousarray(wv_c[:, fs], dtype=np.float32),
            "wp": np.ascontiguousarray(
                w_proj[g * HL * 2 * DV : (g + 1) * HL * 2 * DV, :], dtype=np.float32
            ),
            "bias": (
                np.ascontiguousarray(b_proj, dtype=np.float32)
                if g == 0
                else np.zeros((D,), np.float32)
            ),
        }
        in_maps.append(m)
    return in_maps


def kernel(**inputs) -> np.ndarray:
    from concourse.bass_utils import run_bass_kernel_spmd

    nc = _get_nc()
    in_maps = _shard_inputs(inputs)
    res = run_bass_kernel_spmd(nc, in_maps, list(range(N_CORES)))
    # each core's y holds, per 256-token chunk, its reduce-scatter 128-row
    # half at rows [base + half*128, base + (half+1)*128)
    y = np.empty((B, S, D), np.float32)
    for b in range(B):
        y0 = res.results[2 * b]["y"]
        y1 = res.results[2 * b + 1]["y"]
        for base, w in SLABS:
            cs = min(w, 256)
            for ci in range(max(1, w // 256)):
                tb = base + ci * cs
                hr = cs // 2
                src_r = slice(base + ci * hr, base + (ci + 1) * hr)
                y[b, tb : tb + hr] = y0[src_r]
                y[b, tb + hr : tb + cs] = y1[src_r]
    return y
